# revision 12
# baseline (speedup 1.0000x reference)
"""Trainium2 Bass kernel for nn_BOW_model (embedding-bag mean -> FC -> BatchNorm1d
-> ReLU -> FC -> BCEWithLogits loss).

Self-contained: builds + runs an 8-core SPMD Bass program via
concourse.bass_utils.run_bass_kernel_spmd.

Strategy
--------
Data-parallel over bags. 4096 bags are assigned to 8 cores x 4 windows x 128
bags (greedy-balanced so per-(vocab-chunk, window) token counts are even across
cores; capacities are computed from the actual inputs, so one SPMD program fits
every core). Per core:

  1. dma_gather the core's token embedding rows from the replicated table in
     HBM into SBUF, 128 tokens per tile (token -> partition). int16 gather
     indices limit a single gather to 32768 table rows, so the vocab is
     processed in 4 chunks of 32768 rows. Gathers round-robin over the 4 SWDGE
     queues: each queue's descriptors are generated by a different Q7 core
     pair, which parallelizes descriptor generation 4x.
  2. For each 128-token tile, build the 0/1 matrix P[tok, slot] =
     (slot == bagslot[tok]) on the otherwise-idle Scalar engine
     (a1 = |iota - bagslot|; P = relu(1 - a1), integer-exact), then one PE
     matmul accumulates P.T @ G into the window's PSUM bank. The 1/count
     scaling is applied once per window during the PSUM->SBUF copy.
  3. Stream order is window-major, so each window's transpose (PE) overlaps
     the remaining gathers. Then h_preT = W1T.T @ bowT + b1, partial
     sum/sumsq over the core's bags.
  4. AllReduce (8 cores) of the 4KB partial stats -> global BatchNorm mean/var
     on device; relu(a*h+c) on ACT; logits via M=1 matmuls; stable BCE via
     ln(1+exp(-|l|)); per-core partial loss.

Host work is limited to index preprocessing/sharding, parameter re-layout,
unsharding the logits, and summing the 8 partial losses.
"""

import os
import sys
from contextlib import ExitStack

import numpy as np

try:
    import concourse.bass  # noqa: F401  (present on default PYTHONPATH)
except ImportError:  # pragma: no cover - fallback for bare environments
    sys.path.insert(0, "/opt/trn_rl_repo")

import concourse.bass as bass  # noqa: E402
import concourse.tile as tile  # noqa: E402
from concourse import bacc, library_config, mybir  # noqa: E402
from concourse.bass_utils import run_bass_kernel_spmd  # noqa: E402

F32 = mybir.dt.float32
F32R = mybir.dt.float32r
BF16 = mybir.dt.bfloat16
I16 = mybir.dt.int16
ALU = mybir.AluOpType
ACT = mybir.ActivationFunctionType

# Problem constants (full-size problem; dev harnesses may pass a custom cfg).
FULL_CFG = dict(
    B=4096,          # bags
    V=100000,        # vocab
    D=512,           # hidden dim (must be 512: 4 slices of 128)
    EPS=1e-5,
    NCORES=8,
    NWIN=4,          # windows (128-bag groups) per core; B == NCORES*NWIN*128
    CHUNK_BITS=15,   # gather chunk = 32768 rows (int16 index limit)
    BLOCK_TILES=8,   # 128-token tiles per dma_gather call
    NQUEUES=4,       # SWDGE queues used round-robin by the gathers
    F32R=True,       # fp32r (1 cyc/row) vs fp32 (4 cyc/row) matmuls
    BF16=False,      # gather/convolve the embedding table in bf16
)

WIN = 128
TILE = 128


# --------------------------------------------------------------------------
# Host-side layout
# --------------------------------------------------------------------------

def build_layout(token_ids, segment_ids, cfg):
    """Compute the sharded token layout from the actual inputs.

    Returns a dict with per-core device arrays and the uniform capacities
    (program structure) shared by all cores.
    """
    B, V = cfg["B"], cfg["V"]
    ncores, nwin = cfg["NCORES"], cfg["NWIN"]
    chunk_bits = cfg["CHUNK_BITS"]
    chunk = 1 << chunk_bits
    nchunk = (V + chunk - 1) >> chunk_bits
    nslot = ncores * nwin
    assert B == nslot * WIN, "bag count must exactly fill cores*windows*128"

    tok = np.asarray(token_ids).astype(np.int64).ravel()
    seg = np.asarray(segment_ids).astype(np.int64).ravel()
    T = tok.shape[0]

    counts = np.bincount(seg, minlength=B)
    tok_chunk = tok >> chunk_bits
    # per-bag, per-chunk token counts (for balancing)
    bc = np.bincount(seg * nchunk + tok_chunk, minlength=B * nchunk).reshape(
        B, nchunk
    ).astype(np.int64)

    # Greedy: assign bags (largest first) to the (core,window) slot that
    # minimizes the resulting max normalized per-chunk load.
    order = np.argsort(-counts, kind="stable")
    slot_load = np.zeros((nslot, nchunk), np.float64)
    slot_n = np.zeros(nslot, np.int64)
    bag_slot = np.empty(B, np.int64)
    bag_pos = np.empty(B, np.int64)
    mean_c = np.maximum(bc.sum(axis=0) / nslot, 1.0)  # target per-slot load
    for b in order:
        avail = slot_n < WIN
        cand = (slot_load[avail] + bc[b]) / mean_c
        score = cand.max(axis=1) + 1e-4 * cand.sum(axis=1)
        s = np.flatnonzero(avail)[int(np.argmin(score))]
        bag_slot[b] = s
        bag_pos[b] = slot_n[s]
        slot_n[s] += 1
        slot_load[s] += bc[b]
    assert (slot_n == WIN).all()

    bag_core = bag_slot // nwin
    bag_win = bag_slot % nwin

    tok_core = bag_core[seg]
    tok_win = bag_win[seg]
    tok_pos = bag_pos[seg]                       # slot position within window
    tok_within = (tok & (chunk - 1)).astype(np.int64)

    # group key = (core, window, chunk) -- WINDOW-MAJOR stream order, so each
    # window's epilogue can overlap the remaining gathers.
    gkey = (tok_core * nwin + tok_win) * nchunk + tok_chunk
    gcnt = np.bincount(gkey, minlength=ncores * nwin * nchunk).reshape(
        ncores, nwin, nchunk
    )
    caps = gcnt.max(axis=0)                      # [nwin, nchunk]
    caps = ((caps + TILE - 1) // TILE) * TILE    # round to whole tiles
    ntok_cap = int(caps.sum())
    n_tiles = ntok_cap // TILE

    # group base offsets in the per-core token stream (window-major)
    gbase = np.zeros((nwin, nchunk), np.int64)
    off = 0
    for w in range(nwin):
        for c in range(nchunk):
            gbase[w, c] = off
            off += int(caps[w, c])

    # position of each token in its core's stream
    sort_idx = np.lexsort((np.arange(T), gkey))  # stable by group
    sorted_gkey = gkey[sort_idx]
    grp_start = np.zeros(ncores * nwin * nchunk + 1, np.int64)
    np.cumsum(np.bincount(sorted_gkey, minlength=ncores * nwin * nchunk),
              out=grp_start[1:])
    rank = np.arange(T) - grp_start[sorted_gkey]
    wn = (sorted_gkey // nchunk) % nwin
    ck = sorted_gkey % nchunk
    tok_stream_pos = np.empty(T, np.int64)
    tok_stream_pos[sort_idx] = gbase[wn, ck] + rank

    # fill per-core buffers
    idx_buf = np.zeros((ncores, ntok_cap), np.int16)
    slot_buf = np.full((ncores, ntok_cap), -1.0, np.float32)
    idx_buf[tok_core, tok_stream_pos] = tok_within.astype(np.int16)
    slot_buf[tok_core, tok_stream_pos] = tok_pos.astype(np.float32)

    # device layouts
    # idx: token j at [j%16, j//16], replicated over the 8 groups of 16 rows
    idx_dev = np.tile(
        idx_buf.reshape(ncores, ntok_cap // 16, 16).transpose(0, 2, 1), (1, 8, 1)
    ).copy()                                              # [ncores,128,ntok/16]
    slot_dev = slot_buf.reshape(
        ncores, n_tiles, TILE).transpose(0, 2, 1).copy()  # [ncores,128,n_tiles]
    negslot_dev = -slot_dev

    # per-(window, slot) 1/count, fp32 exact: [ncores, 128, nwin]
    invc_win = np.ones((ncores, WIN, nwin), np.float32)
    invc_win[bag_core, bag_pos, bag_win] = (
        1.0 / np.maximum(counts, 1.0)).astype(np.float32)

    return dict(
        caps=caps, nchunk=nchunk, chunk=chunk, ntok_cap=ntok_cap,
        n_tiles=n_tiles, idx_dev=idx_dev, negslot_dev=negslot_dev,
        slot_dev=slot_dev, invc_win=invc_win, bag_core=bag_core,
        bag_win=bag_win,
        bag_pos=bag_pos,
    )


def param_maps(inputs, layout, cfg):
    """Per-core input maps (device tensors) from full inputs + layout."""
    D, nwin = cfg["D"], cfg["NWIN"]
    ncores = cfg["NCORES"]
    njs = D // 128
    emb = np.asarray(inputs["emb"], np.float32)
    if cfg.get("BF16"):
        import ml_dtypes
        emb = emb.astype(ml_dtypes.bfloat16)
    emb = np.ascontiguousarray(emb)
    W1 = np.asarray(inputs["W1"], np.float32)
    w1t = np.ascontiguousarray(W1.T)                       # [d, j]
    b1c = np.asarray(inputs["b1"], np.float32).reshape(njs, 128).T.copy()
    gammac = np.asarray(inputs["gamma"], np.float32).reshape(njs, 128).T.copy()
    betac = np.asarray(inputs["beta"], np.float32).reshape(njs, 128).T.copy()
    w2c = np.asarray(inputs["W2"], np.float32).reshape(njs, 128).T.copy()
    b2c = np.asarray(inputs["b2"], np.float32).reshape(1, 1)
    t_full = np.asarray(inputs["t"], np.float32)

    iota_c = np.tile(np.arange(128, dtype=np.float32), (128, 1))
    ident_c = np.eye(128, dtype=np.float32)

    bag_core = layout["bag_core"]
    bag_win = layout["bag_win"]
    bag_pos = layout["bag_pos"]

    maps = []
    for c in range(ncores):
        sel = bag_core == c
        tvec = np.zeros((1, nwin * WIN), np.float32)
        tvec[0, bag_win[sel] * WIN + bag_pos[sel]] = t_full[sel]
        maps.append(dict(
            emb=emb,
            idx=layout["idx_dev"][c],
            negslot=layout["negslot_dev"][c],
            posslot=layout["slot_dev"][c],
            invcw=layout["invc_win"][c],
            w1t=w1t, b1c=b1c, gammac=gammac, betac=betac,
            w2c=w2c, b2c=b2c, tvec=tvec,
            iota=iota_c, ident=ident_c,
        ))
    return maps


# --------------------------------------------------------------------------
# Device program
# --------------------------------------------------------------------------

def build_program(layout, cfg, collective=True):
    B, V, D = cfg["B"], cfg["V"], cfg["D"]
    ncores, nwin = cfg["NCORES"], cfg["NWIN"]
    chunk = layout["chunk"]
    nchunk = layout["nchunk"]
    caps = layout["caps"]                 # [nwin, nchunk]
    ntok_cap = layout["ntok_cap"]
    n_tiles = layout["n_tiles"]
    block_tiles = cfg["BLOCK_TILES"]
    nqueues = cfg.get("NQUEUES", 1)
    njs = D // 128
    nds = D // 128
    nbag = nwin * WIN                     # bags per core (free axis of h)
    mm = F32R if cfg.get("F32R") else F32  # h-chain matmul operand dtype
    gdt = BF16 if cfg.get("BF16") else mm  # gather/P matmul operand dtype

    nc = bacc.Bacc("TRN2", target_bir_lowering=False, debug=True,
                   num_devices=ncores, num_swdge_queues=nqueues)

    emb = nc.declare_dram_parameter("emb", [V, D], gdt, isOutput=False)
    idx = nc.declare_dram_parameter("idx", [128, ntok_cap // 16], I16, False)
    negslot = nc.declare_dram_parameter("negslot", [128, n_tiles], F32, False)
    posslot = nc.declare_dram_parameter("posslot", [128, n_tiles], F32, False)
    invcw = nc.declare_dram_parameter("invcw", [128, nwin], F32, False)
    w1t = nc.declare_dram_parameter("w1t", [D, D], mm, False)
    b1c = nc.declare_dram_parameter("b1c", [128, njs], F32, False)
    gammac = nc.declare_dram_parameter("gammac", [128, njs], F32, False)
    betac = nc.declare_dram_parameter("betac", [128, njs], F32, False)
    w2c = nc.declare_dram_parameter("w2c", [128, njs], mm, False)
    b2c = nc.declare_dram_parameter("b2c", [1, 1], F32, False)
    tvec = nc.declare_dram_parameter("tvec", [1, nbag], F32, False)
    iota_in = nc.declare_dram_parameter("iota", [128, 128], F32, False)
    ident_in = nc.declare_dram_parameter("ident", [128, 128], F32, False)

    logits_o = nc.declare_dram_parameter("logits", [1, nbag], F32, isOutput=True)
    loss_o = nc.declare_dram_parameter("loss_part", [1, 1], F32, isOutput=True)
    stats_o = nc.declare_dram_parameter("stats", [128, 2 * njs], F32,
                                        isOutput=True)

    with TileKernel(nc) as (tc, ctx):
        nc.gpsimd.load_library(library_config.mlp)
        cpool = ctx.enter_context(tc.tile_pool(name="const", bufs=1))
        gpool = ctx.enter_context(tc.tile_pool(name="g", bufs=6))
        ppool = ctx.enter_context(tc.tile_pool(name="p", bufs=4))
        spool = ctx.enter_context(tc.tile_pool(name="small", bufs=1))
        bow_ps = ctx.enter_context(
            tc.tile_pool(name="bowps", bufs=nwin, space="PSUM"))
        aux_ps = ctx.enter_context(tc.tile_pool(name="auxps", bufs=2,
                                                space="PSUM"))
        h_ps = ctx.enter_context(tc.tile_pool(name="hps", bufs=2, space="PSUM"))
        hnp = ctx.enter_context(tc.tile_pool(name="hnp", bufs=2))
        dram = ctx.enter_context(tc.tile_pool(name="dram", bufs=1,
                                              space="DRAM"))

        # ---- resident loads ------------------------------------------------
        def load(name_ap, shape, dtype=F32):
            t = cpool.tile(shape, dtype, name=f"ld_{name_ap.name}",
                           tag=f"ld_{name_ap.name}")
            nc.sync.dma_start(out=t[:], in_=name_ap[:])
            return t

        idx_sb = load(idx, [128, ntok_cap // 16], I16)
        nslot_sb = load(negslot, [128, n_tiles])
        pslot_sb = load(posslot, [128, n_tiles])
        invcw_sb = load(invcw, [128, nwin])
        iota_sb = load(iota_in, [128, 128])
        ident_sb = load(ident_in, [128, 128])
        b1_sb = load(b1c, [128, njs])
        gam_sb = load(gammac, [128, njs])
        bet_sb = load(betac, [128, njs])
        w2_sb = load(w2c, [128, njs], mm)
        b2_sb = load(b2c, [1, 1])
        t_sb = load(tvec, [1, nbag])
        # W1T as nds tiles of [128 d, D j]
        w1t_sb = []
        for ds in range(nds):
            w = cpool.tile([128, D], mm, tag=f"w1t{ds}", name=f"w1t{ds}")
            nc.sync.dma_start(out=w[:], in_=w1t[ds * 128:(ds + 1) * 128, :])
            w1t_sb.append(w)

        bowT_sb = [spool.tile([128, nbag], mm, tag=f"bowT{ds}",
                              name=f"bowT{ds}")
                   for ds in range(nds)]

        # ---- main loop: gather + P-matmul accumulate (window-major) --------
        gather_cnt = 0
        gtile_idx = 0
        for w in range(nwin):
            bow = bow_ps.tile([128, D], F32, tag="bow", name=f"bow{w}")
            wtiles = int(caps[w].sum()) // TILE
            wseen = 0
            for c in range(nchunk):
                ctiles = int(caps[w][c]) // TILE
                if ctiles == 0:
                    continue
                base_row = c * chunk
                rows = min(chunk, V - base_row)
                done = 0
                while done < ctiles:
                    bt = min(block_tiles, ctiles - done)
                    g = gpool.tile([128, bt, D], gdt, tag="g")
                    col0 = (gtile_idx + done) * (TILE // 16)
                    nc.gpsimd.dma_gather(
                        out_ap=g[:],
                        in_ap=emb[base_row:base_row + rows, :],
                        idxs_ap=idx_sb[:, col0:col0 + bt * (TILE // 16)],
                        num_idxs=bt * TILE,
                        num_idxs_reg=bt * TILE,
                        elem_size=D,
                        queue_num=gather_cnt % nqueues,
                    )
                    gather_cnt += 1
                    for k in range(bt):
                        ti = gtile_idx + done + k
                        # P[tok, slot] = (slot == bagslot[tok]), integer-exact.
                        # "dve": one is_equal against the iota tile.
                        # "act": a1 = |iota - bagslot|; P = relu(1 - a1) on
                        #        the Scalar engine. "mix" alternates.
                        p = ppool.tile([128, 128], gdt, tag="p")
                        pb = cfg.get("PBUILD", "dve")
                        if pb == "mix":
                            pb = "dve" if ti % 2 == 0 else "act"
                        if pb == "act":
                            a1 = ppool.tile([128, 128], F32, tag="a1",
                                            name=f"a1_{ti}")
                            nc.scalar.activation(a1[:], iota_sb[:], ACT.Abs,
                                                 bias=nslot_sb[:, ti:ti + 1])
                            nc.scalar.activation(p[:], a1[:], ACT.Relu,
                                                 scale=-1.0, bias=1.0)
                        else:
                            nc.vector.tensor_scalar(
                                p[:], iota_sb[:], pslot_sb[:, ti:ti + 1],
                                None, ALU.is_equal)
                        nc.tensor.matmul(
                            bow[:], p[:], g[:, k, :],
                            start=(wseen == 0),
                            stop=(wseen == wtiles - 1))
                        wseen += 1
                    done += bt
                gtile_idx += ctiles

            # ---- window epilogue: scale by 1/count, transpose ---------------
            bsb = spool.tile([128, D], F32, tag="bow_sb", name=f"bow_sb{w}",
                             bufs=2)
            nc.vector.tensor_scalar_mul(bsb[:], bow[:], invcw_sb[:, w:w + 1])
            for ds in range(nds):
                tp = aux_ps.tile([128, 128], F32, tag="tp")
                nc.tensor.transpose(tp[:], bsb[:, ds * 128:(ds + 1) * 128],
                                    ident_sb[:])
                nc.vector.tensor_copy(bowT_sb[ds][:, w * WIN:(w + 1) * WIN],
                                      tp[:])

        # ---- h_preT = W1T.T @ bowT + b1; partial stats ---------------------
        h_sb = []
        s_sb = spool.tile([128, 2 * njs], F32, tag="s12")
        for js in range(njs):
            hp = h_ps.tile([128, nbag], F32, tag="hps")
            for ds in range(nds):
                nc.tensor.matmul(
                    hp[:],
                    w1t_sb[ds][:, js * 128:(js + 1) * 128],
                    bowT_sb[ds][:],
                    start=(ds == 0), stop=(ds == nds - 1))
            h = spool.tile([128, nbag], F32, tag=f"h{js}")
            nc.vector.tensor_scalar_add(h[:], hp[:], b1_sb[:, js:js + 1])
            h_sb.append(h)
            nc.vector.tensor_reduce(s_sb[:, js:js + 1], h[:],
                                    mybir.AxisListType.X, ALU.add)
            sq = spool.tile([128, nbag], F32, tag="sq")
            nc.vector.tensor_mul(sq[:], h[:], h[:])
            nc.vector.tensor_reduce(s_sb[:, njs + js:njs + js + 1], sq[:],
                                    mybir.AxisListType.X, ALU.add)

        # ---- global stats (AllReduce) --------------------------------------
        if collective and ncores > 1:
            cc_in = dram.tile([128, 2 * njs], F32)
            cc_out = dram.tile([128, 2 * njs], F32)
            nc.sync.dma_start(out=cc_in[:], in_=s_sb[:])
            nc.gpsimd.collective_compute(
                "AllReduce", ALU.add,
                replica_groups=[list(range(ncores))],
                ins=[cc_in.opt()], outs=[cc_out.opt()])
            sg_sb = spool.tile([128, 2 * njs], F32, tag="sg")
            nc.sync.dma_start(out=sg_sb[:], in_=cc_out[:])
        else:
            sg_sb = s_sb
        nc.sync.dma_start(out=stats_o[:], in_=sg_sb[:])

        mu = spool.tile([128, njs], F32, tag="mu")
        nc.vector.tensor_scalar(mu[:], sg_sb[:, 0:njs], 1.0 / B, None, ALU.mult)
        ex2 = spool.tile([128, njs], F32, tag="ex2")
        nc.vector.tensor_scalar(ex2[:], sg_sb[:, njs:2 * njs], 1.0 / B, None,
                                ALU.mult)
        mu2 = spool.tile([128, njs], F32, tag="mu2")
        nc.vector.tensor_mul(mu2[:], mu[:], mu[:])
        var = spool.tile([128, njs], F32, tag="var")
        nc.vector.scalar_tensor_tensor(var[:], mu2[:], -1.0, ex2[:],
                                       ALU.mult, ALU.add)
        vpe = spool.tile([128, njs], F32, tag="vpe")
        nc.vector.tensor_scalar_add(vpe[:], var[:], float(cfg["EPS"]))
        sd = spool.tile([128, njs], F32, tag="sd")
        nc.scalar.activation(sd[:], vpe[:], ACT.Sqrt)
        rstd = spool.tile([128, njs], F32, tag="rstd")
        nc.vector.reciprocal(rstd[:], sd[:])
        a_sb = spool.tile([128, njs], F32, tag="a")
        nc.vector.tensor_mul(a_sb[:], gam_sb[:], rstd[:])
        mua = spool.tile([128, njs], F32, tag="mua")
        nc.vector.tensor_mul(mua[:], mu[:], a_sb[:])
        c_sb = spool.tile([128, njs], F32, tag="c")
        nc.vector.scalar_tensor_tensor(c_sb[:], mua[:], -1.0, bet_sb[:],
                                       ALU.mult, ALU.add)

        # ---- normalize + relu + logits -------------------------------------
        # reuse the transpose slots (all 8 PSUM banks accounted: 4 bow + 2 tp
        # + 2 h)
        lg_ps = aux_ps.tile([1, nbag], F32, tag="tp")
        for js in range(njs):
            hn = hnp.tile([128, nbag], mm, tag="hn", name=f"hn{js}")
            nc.scalar.activation(hn[:], h_sb[js][:], ACT.Relu,
                                 scale=a_sb[:, js:js + 1],
                                 bias=c_sb[:, js:js + 1])
            nc.tensor.matmul(lg_ps[:], w2_sb[:, js:js + 1], hn[:],
                             start=(js == 0), stop=(js == njs - 1))
        lg = spool.tile([1, nbag], F32, tag="lg")
        nc.vector.tensor_scalar_add(lg[:], lg_ps[:], b2_sb[0:1, 0:1])
        nc.sync.dma_start(out=logits_o[:], in_=lg[:])

        # ---- BCE loss partial ----------------------------------------------
        m = spool.tile([1, nbag], F32, tag="m")
        nc.vector.tensor_scalar_max(m[:], lg[:], 0.0)
        lt = spool.tile([1, nbag], F32, tag="lt")
        nc.vector.tensor_mul(lt[:], lg[:], t_sb[:])
        al = spool.tile([1, nbag], F32, tag="al")
        nc.scalar.activation(al[:], lg[:], ACT.Abs)
        u = spool.tile([1, nbag], F32, tag="u")
        nc.scalar.activation(u[:], al[:], ACT.Exp, scale=-1.0)
        up1 = spool.tile([1, nbag], F32, tag="up1")
        nc.vector.tensor_scalar_add(up1[:], u[:], 1.0)
        sp = spool.tile([1, nbag], F32, tag="sp")
        nc.scalar.activation(sp[:], up1[:], ACT.Ln)
        mml = spool.tile([1, nbag], F32, tag="mml")
        nc.vector.scalar_tensor_tensor(mml[:], lt[:], -1.0, m[:],
                                       ALU.mult, ALU.add)
        lv = spool.tile([1, nbag], F32, tag="lv")
        nc.vector.tensor_add(lv[:], mml[:], sp[:])
        lp = spool.tile([1, 1], F32, tag="lp")
        nc.vector.tensor_reduce(lp[:], lv[:], mybir.AxisListType.X, ALU.add)
        lps = spool.tile([1, 1], F32, tag="lps")
        nc.vector.tensor_scalar(lps[:], lp[:], 1.0 / B, None, ALU.mult)
        nc.sync.dma_start(out=loss_o[:], in_=lps[:])

    nc.compile()
    return nc


class TileKernel:
    """Small helper: `with TileKernel(nc) as (tc, ctx):`"""

    def __init__(self, nc):
        self.nc = nc

    def __enter__(self):
        self.ctx = ExitStack()
        self.tc = self.ctx.enter_context(tile.TileContext(self.nc))
        return self.tc, self.ctx

    def __exit__(self, *exc):
        return self.ctx.__exit__(*exc)


# --------------------------------------------------------------------------
# Entry point
# --------------------------------------------------------------------------

def run(inputs, cfg=None, collective=True, trace=False, trace_kwargs=None):
    cfg = dict(FULL_CFG if cfg is None else cfg)
    layout = build_layout(inputs["token_ids"], inputs["segment_ids"], cfg)
    nc = build_program(layout, cfg, collective=collective)
    maps = param_maps(inputs, layout, cfg)
    res = run_bass_kernel_spmd(
        nc, maps, list(range(cfg["NCORES"])),
        trace=trace, **(trace_kwargs or {}))

    ncores = cfg["NCORES"]
    B = cfg["B"]
    logits = np.zeros(B, np.float32)
    loss = np.float64(0.0)
    for c in range(ncores):
        out = res.results[c]
        core_bags = np.flatnonzero(layout["bag_core"] == c)
        pos = (layout["bag_win"][core_bags] * WIN
               + layout["bag_pos"][core_bags])
        logits[core_bags] = out["logits"][0, pos]
        loss += np.float64(out["loss_part"][0, 0])
    return (np.float32(loss), logits), res


def kernel(**inputs):
    (loss, logits), _ = run(inputs)
    return (loss, logits)


# revision 16
# speedup vs baseline: 1.6843x; 1.6843x over previous
"""Trainium2 Bass kernel for nn_BOW_model (embedding-bag mean -> FC -> BatchNorm1d
-> ReLU -> FC -> BCEWithLogits loss).

Self-contained: builds + runs an 8-core SPMD Bass program via
concourse.bass_utils.run_bass_kernel_spmd.

Strategy
--------
Data-parallel over bags. 4096 bags are assigned to 8 cores x 4 windows x 128
bags (greedy-balanced so per-(vocab-chunk, window) token counts are even across
cores; capacities are computed from the actual inputs, so one SPMD program fits
every core). Per core:

  1. dma_gather the core's token embedding rows from the replicated table in
     HBM into SBUF, 128 tokens per tile (token -> partition). int16 gather
     indices limit a single gather to 32768 table rows, so the vocab is
     processed in 4 chunks of 32768 rows. Gathers round-robin over the 4 SWDGE
     queues: each queue's descriptors are generated by a different Q7 core
     pair, which parallelizes descriptor generation 4x.
  2. For each 128-token tile, build the 0/1 matrix P[tok, slot] =
     (slot == bagslot[tok]) on the otherwise-idle Scalar engine
     (a1 = |iota - bagslot|; P = relu(1 - a1), integer-exact), then one PE
     matmul accumulates P.T @ G into the window's PSUM bank. The 1/count
     scaling is applied once per window during the PSUM->SBUF copy.
  3. Stream order is window-major, so each window's transpose (PE) overlaps
     the remaining gathers. Then h_preT = W1T.T @ bowT + b1, partial
     sum/sumsq over the core's bags.
  4. AllReduce (8 cores) of the 4KB partial stats -> global BatchNorm mean/var
     on device; relu(a*h+c) on ACT; logits via M=1 matmuls; stable BCE via
     ln(1+exp(-|l|)); per-core partial loss.

Host work is limited to index preprocessing/sharding, parameter re-layout,
unsharding the logits, and summing the 8 partial losses.
"""

import os
import sys
from contextlib import ExitStack

import numpy as np

try:
    import concourse.bass  # noqa: F401  (present on default PYTHONPATH)
except ImportError:  # pragma: no cover - fallback for bare environments
    sys.path.insert(0, "/opt/trn_rl_repo")

import concourse.bass as bass  # noqa: E402
import concourse.tile as tile  # noqa: E402
from concourse import bacc, library_config, mybir  # noqa: E402
from concourse.bass_utils import run_bass_kernel_spmd  # noqa: E402

F32 = mybir.dt.float32
F32R = mybir.dt.float32r
BF16 = mybir.dt.bfloat16
I16 = mybir.dt.int16
ALU = mybir.AluOpType
ACT = mybir.ActivationFunctionType

# Problem constants (full-size problem; dev harnesses may pass a custom cfg).
FULL_CFG = dict(
    B=4096,          # bags
    V=100000,        # vocab
    D=512,           # hidden dim (must be 512: 4 slices of 128)
    EPS=1e-5,
    NCORES=8,
    NWIN=4,          # windows (128-bag groups) per core; B == NCORES*NWIN*128
    CHUNK_BITS=15,   # gather chunk = 32768 rows (int16 index limit)
    BLOCK_TILES=8,   # 128-token tiles per dma_gather call
    NQUEUES=4,       # SWDGE queues used round-robin by the gathers
    F32R=True,       # fp32r (1 cyc/row) vs fp32 (4 cyc/row) matmuls
    BF16=False,      # gather/convolve the embedding table in bf16
)

WIN = 128
TILE = 128


# --------------------------------------------------------------------------
# Host-side layout
# --------------------------------------------------------------------------

def build_layout(token_ids, segment_ids, cfg):
    """Compute the sharded token layout from the actual inputs.

    Returns a dict with per-core device arrays and the uniform capacities
    (program structure) shared by all cores.
    """
    B, V = cfg["B"], cfg["V"]
    ncores, nwin = cfg["NCORES"], cfg["NWIN"]
    chunk_bits = cfg["CHUNK_BITS"]
    chunk = 1 << chunk_bits
    nchunk = (V + chunk - 1) >> chunk_bits
    nslot = ncores * nwin
    assert B == nslot * WIN, "bag count must exactly fill cores*windows*128"

    tok = np.asarray(token_ids).astype(np.int64).ravel()
    seg = np.asarray(segment_ids).astype(np.int64).ravel()
    T = tok.shape[0]

    counts = np.bincount(seg, minlength=B)
    tok_chunk = tok >> chunk_bits
    # per-bag, per-chunk token counts (for balancing)
    bc = np.bincount(seg * nchunk + tok_chunk, minlength=B * nchunk).reshape(
        B, nchunk
    ).astype(np.int64)

    # Greedy: assign bags (largest first) to the (core,window) slot that
    # minimizes the resulting max normalized per-chunk load.
    order = np.argsort(-counts, kind="stable")
    slot_load = np.zeros((nslot, nchunk), np.float64)
    slot_n = np.zeros(nslot, np.int64)
    bag_slot = np.empty(B, np.int64)
    bag_pos = np.empty(B, np.int64)
    mean_c = np.maximum(bc.sum(axis=0) / nslot, 1.0)  # target per-slot load
    for b in order:
        avail = slot_n < WIN
        cand = (slot_load[avail] + bc[b]) / mean_c
        score = cand.max(axis=1) + 1e-4 * cand.sum(axis=1)
        s = np.flatnonzero(avail)[int(np.argmin(score))]
        bag_slot[b] = s
        bag_pos[b] = slot_n[s]
        slot_n[s] += 1
        slot_load[s] += bc[b]
    assert (slot_n == WIN).all()

    bag_core = bag_slot // nwin
    bag_win = bag_slot % nwin

    tok_core = bag_core[seg]
    tok_win = bag_win[seg]
    tok_pos = bag_pos[seg]                       # slot position within window
    tok_within = (tok & (chunk - 1)).astype(np.int64)

    # group key = (core, window, chunk) -- WINDOW-MAJOR stream order, so each
    # window's epilogue can overlap the remaining gathers.
    gkey = (tok_core * nwin + tok_win) * nchunk + tok_chunk
    gcnt = np.bincount(gkey, minlength=ncores * nwin * nchunk).reshape(
        ncores, nwin, nchunk
    )
    caps = gcnt.max(axis=0)                      # [nwin, nchunk]
    caps = ((caps + TILE - 1) // TILE) * TILE    # round to whole tiles
    ntok_cap = int(caps.sum())
    n_tiles = ntok_cap // TILE

    # group base offsets in the per-core token stream (window-major)
    gbase = np.zeros((nwin, nchunk), np.int64)
    off = 0
    for w in range(nwin):
        for c in range(nchunk):
            gbase[w, c] = off
            off += int(caps[w, c])

    # position of each token in its core's stream
    sort_idx = np.lexsort((np.arange(T), gkey))  # stable by group
    sorted_gkey = gkey[sort_idx]
    grp_start = np.zeros(ncores * nwin * nchunk + 1, np.int64)
    np.cumsum(np.bincount(sorted_gkey, minlength=ncores * nwin * nchunk),
              out=grp_start[1:])
    rank = np.arange(T) - grp_start[sorted_gkey]
    wn = (sorted_gkey // nchunk) % nwin
    ck = sorted_gkey % nchunk
    tok_stream_pos = np.empty(T, np.int64)
    tok_stream_pos[sort_idx] = gbase[wn, ck] + rank

    # fill per-core buffers
    idx_buf = np.zeros((ncores, ntok_cap), np.int16)
    slot_buf = np.full((ncores, ntok_cap), 1000.0, np.float32)
    idx_buf[tok_core, tok_stream_pos] = tok_within.astype(np.int16)
    slot_buf[tok_core, tok_stream_pos] = tok_pos.astype(np.float32)

    # device layouts
    # idx: token j at [j%16, j//16], replicated over the 8 groups of 16 rows
    idx_dev = np.tile(
        idx_buf.reshape(ncores, ntok_cap // 16, 16).transpose(0, 2, 1), (1, 8, 1)
    ).copy()                                              # [ncores,128,ntok/16]
    slot_dev = slot_buf.reshape(
        ncores, n_tiles, TILE).transpose(0, 2, 1).copy()  # [ncores,128,n_tiles]
    negslot_dev = -slot_dev
    stepbias_dev = 60.0 * (0.5 - slot_dev)   # sigmoid-step bias, K=60

    # per-(window, slot) 1/count, fp32 exact: [ncores, 128, nwin]
    invc_win = np.ones((ncores, WIN, nwin), np.float32)
    invc_win[bag_core, bag_pos, bag_win] = (
        1.0 / np.maximum(counts, 1.0)).astype(np.float32)
    # Transpose-matmul moving operand, one per window: recovers per-slot sums
    # from the cumulative-over-slot PSUM (difference) and applies 1/count, all
    # inside the existing PE transpose: dmat[w][p, j] = invc[j]*(d(p==j) -
    # d(p==j-1)).
    base = np.eye(WIN, dtype=np.float32) - np.eye(WIN, k=1, dtype=np.float32)
    dmat = np.empty((ncores, nwin, WIN, WIN), np.float32)
    for c in range(ncores):
        for w in range(nwin):
            dmat[c, w] = base * invc_win[c, :, w][None, :]

    return dict(
        caps=caps, nchunk=nchunk, chunk=chunk, ntok_cap=ntok_cap,
        n_tiles=n_tiles, idx_dev=idx_dev, negslot_dev=negslot_dev,
        slot_dev=slot_dev, stepbias_dev=stepbias_dev, invc_win=invc_win,
        dmat=dmat, bag_core=bag_core, bag_win=bag_win,
        bag_pos=bag_pos,
    )


def param_maps(inputs, layout, cfg):
    """Per-core input maps (device tensors) from full inputs + layout."""
    D, nwin = cfg["D"], cfg["NWIN"]
    ncores = cfg["NCORES"]
    njs = D // 128
    emb = np.asarray(inputs["emb"], np.float32)
    if cfg.get("BF16"):
        import ml_dtypes
        emb = emb.astype(ml_dtypes.bfloat16)
    emb = np.ascontiguousarray(emb)
    W1 = np.asarray(inputs["W1"], np.float32)
    w1t = np.ascontiguousarray(W1.T)                       # [d, j]
    b1c = np.asarray(inputs["b1"], np.float32).reshape(njs, 128).T.copy()
    gammac = np.asarray(inputs["gamma"], np.float32).reshape(njs, 128).T.copy()
    betac = np.asarray(inputs["beta"], np.float32).reshape(njs, 128).T.copy()
    w2c = np.asarray(inputs["W2"], np.float32).reshape(njs, 128).T.copy()
    b2c = np.asarray(inputs["b2"], np.float32).reshape(1, 1)
    t_full = np.asarray(inputs["t"], np.float32)

    iota_c = np.tile(np.arange(128, dtype=np.float32), (128, 1))

    bag_core = layout["bag_core"]
    bag_win = layout["bag_win"]
    bag_pos = layout["bag_pos"]

    maps = []
    for c in range(ncores):
        sel = bag_core == c
        tvec = np.zeros((1, nwin * WIN), np.float32)
        tvec[0, bag_win[sel] * WIN + bag_pos[sel]] = t_full[sel]
        maps.append(dict(
            emb=emb,
            idx=layout["idx_dev"][c],
            negslot=layout["negslot_dev"][c],
            posslot=layout["slot_dev"][c],
            stepbias=layout["stepbias_dev"][c],
            w1t=w1t, b1c=b1c, gammac=gammac, betac=betac,
            w2c=w2c, b2c=b2c, tvec=tvec,
            iota=iota_c, dmat=layout["dmat"][c],
        ))
    return maps


# --------------------------------------------------------------------------
# Device program
# --------------------------------------------------------------------------

def build_program(layout, cfg, collective=True):
    B, V, D = cfg["B"], cfg["V"], cfg["D"]
    ncores, nwin = cfg["NCORES"], cfg["NWIN"]
    chunk = layout["chunk"]
    nchunk = layout["nchunk"]
    caps = layout["caps"]                 # [nwin, nchunk]
    ntok_cap = layout["ntok_cap"]
    n_tiles = layout["n_tiles"]
    block_tiles = cfg["BLOCK_TILES"]
    nqueues = cfg.get("NQUEUES", 1)
    njs = D // 128
    nds = D // 128
    nbag = nwin * WIN                     # bags per core (free axis of h)
    mm = F32R if cfg.get("F32R") else F32  # h-chain matmul operand dtype
    gdt = BF16 if cfg.get("BF16") else mm  # gather/P matmul operand dtype

    nc = bacc.Bacc("TRN2", target_bir_lowering=False, debug=True,
                   num_devices=ncores, num_swdge_queues=nqueues)

    emb = nc.declare_dram_parameter("emb", [V, D], gdt, isOutput=False)
    idx = nc.declare_dram_parameter("idx", [128, ntok_cap // 16], I16, False)
    negslot = nc.declare_dram_parameter("negslot", [128, n_tiles], F32, False)
    posslot = nc.declare_dram_parameter("posslot", [128, n_tiles], F32, False)
    stepbias = nc.declare_dram_parameter("stepbias", [128, n_tiles], F32,
                                         False)
    dmat_in = nc.declare_dram_parameter("dmat", [nwin, WIN, WIN], F32, False)
    w1t = nc.declare_dram_parameter("w1t", [D, D], mm, False)
    b1c = nc.declare_dram_parameter("b1c", [128, njs], F32, False)
    gammac = nc.declare_dram_parameter("gammac", [128, njs], F32, False)
    betac = nc.declare_dram_parameter("betac", [128, njs], F32, False)
    w2c = nc.declare_dram_parameter("w2c", [128, njs], mm, False)
    b2c = nc.declare_dram_parameter("b2c", [1, 1], F32, False)
    tvec = nc.declare_dram_parameter("tvec", [1, nbag], F32, False)
    iota_in = nc.declare_dram_parameter("iota", [128, 128], F32, False)

    logits_o = nc.declare_dram_parameter("logits", [1, nbag], F32, isOutput=True)
    loss_o = nc.declare_dram_parameter("loss_part", [1, 1], F32, isOutput=True)
    stats_o = nc.declare_dram_parameter("stats", [128, 2 * njs], F32,
                                        isOutput=True)

    with TileKernel(nc) as (tc, ctx):
        nc.gpsimd.load_library(library_config.mlp)
        cpool = ctx.enter_context(tc.tile_pool(name="const", bufs=1))
        gpool = ctx.enter_context(tc.tile_pool(name="g", bufs=6))
        ppool = ctx.enter_context(tc.tile_pool(name="p", bufs=4))
        spool = ctx.enter_context(tc.tile_pool(name="small", bufs=1))
        bow_ps = ctx.enter_context(
            tc.tile_pool(name="bowps", bufs=nwin, space="PSUM"))
        aux_ps = ctx.enter_context(tc.tile_pool(name="auxps", bufs=2,
                                                space="PSUM"))
        h_ps = ctx.enter_context(tc.tile_pool(name="hps", bufs=2, space="PSUM"))
        hnp = ctx.enter_context(tc.tile_pool(name="hnp", bufs=2))
        dram = ctx.enter_context(tc.tile_pool(name="dram", bufs=1,
                                              space="DRAM"))

        # ---- resident loads ------------------------------------------------
        def load(name_ap, shape, dtype=F32):
            t = cpool.tile(shape, dtype, name=f"ld_{name_ap.name}",
                           tag=f"ld_{name_ap.name}")
            nc.sync.dma_start(out=t[:], in_=name_ap[:])
            return t

        idx_sb = load(idx, [128, ntok_cap // 16], I16)
        nslot_sb = load(negslot, [128, n_tiles])
        pslot_sb = load(posslot, [128, n_tiles])
        sbias_sb = load(stepbias, [128, n_tiles])
        iota_sb = load(iota_in, [128, 128])
        dmat_sb = []
        for w in range(nwin):
            dm = cpool.tile([128, WIN], F32, tag=f"dmat{w}", name=f"dmat{w}")
            nc.sync.dma_start(out=dm[:], in_=dmat_in[w])
            dmat_sb.append(dm)
        b1_sb = load(b1c, [128, njs])
        gam_sb = load(gammac, [128, njs])
        bet_sb = load(betac, [128, njs])
        w2_sb = load(w2c, [128, njs], mm)
        b2_sb = load(b2c, [1, 1])
        t_sb = load(tvec, [1, nbag])
        # W1T as nds tiles of [128 d, D j]
        w1t_sb = []
        for ds in range(nds):
            w = cpool.tile([128, D], mm, tag=f"w1t{ds}", name=f"w1t{ds}")
            nc.sync.dma_start(out=w[:], in_=w1t[ds * 128:(ds + 1) * 128, :])
            w1t_sb.append(w)

        bowT_sb = [spool.tile([128, nbag], mm, tag=f"bowT{ds}",
                              name=f"bowT{ds}")
                   for ds in range(nds)]

        # ---- main loop: gather + P-matmul accumulate (window-major) --------
        gather_cnt = 0
        gtile_idx = 0
        for w in range(nwin):
            bow = bow_ps.tile([128, D], F32, tag="bow", name=f"bow{w}")
            wtiles = int(caps[w].sum()) // TILE
            wseen = 0
            for c in range(nchunk):
                ctiles = int(caps[w][c]) // TILE
                if ctiles == 0:
                    continue
                base_row = c * chunk
                rows = min(chunk, V - base_row)
                done = 0
                while done < ctiles:
                    bt = min(block_tiles, ctiles - done)
                    g = gpool.tile([128, bt, D], gdt, tag="g")
                    col0 = (gtile_idx + done) * (TILE // 16)
                    nc.gpsimd.dma_gather(
                        out_ap=g[:],
                        in_ap=emb[base_row:base_row + rows, :],
                        idxs_ap=idx_sb[:, col0:col0 + bt * (TILE // 16)],
                        num_idxs=bt * TILE,
                        num_idxs_reg=bt * TILE,
                        elem_size=D,
                        queue_num=gather_cnt % nqueues,
                    )
                    gather_cnt += 1
                    for k in range(bt):
                        ti = gtile_idx + done + k
                        # P[tok, slot] = (slot == bagslot[tok]), integer-exact.
                        # "dve": one is_equal against the iota tile.
                        # "act": a1 = |iota - bagslot|; P = relu(1 - a1) on
                        #        the Scalar engine. "mix" alternates.
                        p = ppool.tile([128, 128], gdt, tag="p")
                        pb = cfg.get("PBUILD", "step")
                        if pb == "mix":
                            pb = "dve" if ti % 2 == 0 else "act"
                        if pb == "step":
                            # S[tok, f] = 1 iff f >= bagslot[tok]: sigmoid
                            # saturates exactly at +-30. The matmul then
                            # accumulates cumulative-over-slot sums; the
                            # window epilogue takes the adjacent difference.
                            nc.scalar.activation(p[:], iota_sb[:], ACT.Sigmoid,
                                                 scale=60.0,
                                                 bias=sbias_sb[:, ti:ti + 1])
                        elif pb == "act":
                            a1 = ppool.tile([128, 128], F32, tag="a1",
                                            name=f"a1_{ti}")
                            nc.scalar.activation(a1[:], iota_sb[:], ACT.Abs,
                                                 bias=nslot_sb[:, ti:ti + 1])
                            nc.scalar.activation(p[:], a1[:], ACT.Relu,
                                                 scale=-1.0, bias=1.0)
                        else:
                            nc.vector.tensor_scalar(
                                p[:], iota_sb[:], pslot_sb[:, ti:ti + 1],
                                None, ALU.is_equal)
                        nc.tensor.matmul(
                            bow[:], p[:], g[:, k, :],
                            start=(wseen == 0),
                            stop=(wseen == wtiles - 1))
                        wseen += 1
                    done += bt
                gtile_idx += ctiles

            # ---- window epilogue -------------------------------------------
            # The PE transpose's moving operand is dmat[w] (not identity): it
            # simultaneously transposes, takes the adjacent difference that
            # turns the cumulative-over-slot PSUM into per-slot sums (step
            # P-build), and applies the per-bag 1/count.
            bsb = spool.tile([128, D], F32, tag="bow_sb", name=f"bow_sb{w}",
                             bufs=2)
            nc.vector.tensor_copy(bsb[:], bow[:])
            for ds in range(nds):
                tp = aux_ps.tile([128, 128], F32, tag="tp")
                nc.tensor.matmul(tp[:], bsb[:, ds * 128:(ds + 1) * 128],
                                 dmat_sb[w][:], start=True, stop=True)
                nc.vector.tensor_copy(bowT_sb[ds][:, w * WIN:(w + 1) * WIN],
                                      tp[:])

        # ---- h_preT = W1T.T @ bowT + b1; partial stats ---------------------
        h_sb = []
        s_sb = spool.tile([128, 2 * njs], F32, tag="s12")
        for js in range(njs):
            hp = h_ps.tile([128, nbag], F32, tag="hps")
            for ds in range(nds):
                nc.tensor.matmul(
                    hp[:],
                    w1t_sb[ds][:, js * 128:(js + 1) * 128],
                    bowT_sb[ds][:],
                    start=(ds == 0), stop=(ds == nds - 1))
            h = spool.tile([128, nbag], F32, tag=f"h{js}")
            nc.vector.tensor_scalar_add(h[:], hp[:], b1_sb[:, js:js + 1])
            h_sb.append(h)
            nc.vector.tensor_reduce(s_sb[:, js:js + 1], h[:],
                                    mybir.AxisListType.X, ALU.add)
            sq = spool.tile([128, nbag], F32, tag="sq")
            nc.vector.tensor_mul(sq[:], h[:], h[:])
            nc.vector.tensor_reduce(s_sb[:, njs + js:njs + js + 1], sq[:],
                                    mybir.AxisListType.X, ALU.add)

        # ---- global stats (AllReduce) --------------------------------------
        if collective and ncores > 1:
            cc_in = dram.tile([128, 2 * njs], F32)
            cc_out = dram.tile([128, 2 * njs], F32)
            nc.sync.dma_start(out=cc_in[:], in_=s_sb[:])
            nc.gpsimd.collective_compute(
                "AllReduce", ALU.add,
                replica_groups=[list(range(ncores))],
                ins=[cc_in.opt()], outs=[cc_out.opt()])
            sg_sb = spool.tile([128, 2 * njs], F32, tag="sg")
            nc.sync.dma_start(out=sg_sb[:], in_=cc_out[:])
        else:
            sg_sb = s_sb
        nc.sync.dma_start(out=stats_o[:], in_=sg_sb[:])

        mu = spool.tile([128, njs], F32, tag="mu")
        nc.vector.tensor_scalar(mu[:], sg_sb[:, 0:njs], 1.0 / B, None, ALU.mult)
        ex2 = spool.tile([128, njs], F32, tag="ex2")
        nc.vector.tensor_scalar(ex2[:], sg_sb[:, njs:2 * njs], 1.0 / B, None,
                                ALU.mult)
        mu2 = spool.tile([128, njs], F32, tag="mu2")
        nc.vector.tensor_mul(mu2[:], mu[:], mu[:])
        var = spool.tile([128, njs], F32, tag="var")
        nc.vector.scalar_tensor_tensor(var[:], mu2[:], -1.0, ex2[:],
                                       ALU.mult, ALU.add)
        vpe = spool.tile([128, njs], F32, tag="vpe")
        nc.vector.tensor_scalar_add(vpe[:], var[:], float(cfg["EPS"]))
        sd = spool.tile([128, njs], F32, tag="sd")
        nc.scalar.activation(sd[:], vpe[:], ACT.Sqrt)
        rstd = spool.tile([128, njs], F32, tag="rstd")
        nc.vector.reciprocal(rstd[:], sd[:])
        a_sb = spool.tile([128, njs], F32, tag="a")
        nc.vector.tensor_mul(a_sb[:], gam_sb[:], rstd[:])
        mua = spool.tile([128, njs], F32, tag="mua")
        nc.vector.tensor_mul(mua[:], mu[:], a_sb[:])
        c_sb = spool.tile([128, njs], F32, tag="c")
        nc.vector.scalar_tensor_tensor(c_sb[:], mua[:], -1.0, bet_sb[:],
                                       ALU.mult, ALU.add)

        # ---- normalize + relu + logits -------------------------------------
        # reuse the transpose slots (all 8 PSUM banks accounted: 4 bow + 2 tp
        # + 2 h)
        lg_ps = aux_ps.tile([1, nbag], F32, tag="tp")
        for js in range(njs):
            hn = hnp.tile([128, nbag], mm, tag="hn", name=f"hn{js}")
            nc.scalar.activation(hn[:], h_sb[js][:], ACT.Relu,
                                 scale=a_sb[:, js:js + 1],
                                 bias=c_sb[:, js:js + 1])
            nc.tensor.matmul(lg_ps[:], w2_sb[:, js:js + 1], hn[:],
                             start=(js == 0), stop=(js == njs - 1))
        lg = spool.tile([1, nbag], F32, tag="lg")
        nc.vector.tensor_scalar_add(lg[:], lg_ps[:], b2_sb[0:1, 0:1])
        nc.sync.dma_start(out=logits_o[:], in_=lg[:])

        # ---- BCE loss partial ----------------------------------------------
        m = spool.tile([1, nbag], F32, tag="m")
        nc.vector.tensor_scalar_max(m[:], lg[:], 0.0)
        lt = spool.tile([1, nbag], F32, tag="lt")
        nc.vector.tensor_mul(lt[:], lg[:], t_sb[:])
        al = spool.tile([1, nbag], F32, tag="al")
        nc.scalar.activation(al[:], lg[:], ACT.Abs)
        u = spool.tile([1, nbag], F32, tag="u")
        nc.scalar.activation(u[:], al[:], ACT.Exp, scale=-1.0)
        up1 = spool.tile([1, nbag], F32, tag="up1")
        nc.vector.tensor_scalar_add(up1[:], u[:], 1.0)
        sp = spool.tile([1, nbag], F32, tag="sp")
        nc.scalar.activation(sp[:], up1[:], ACT.Ln)
        mml = spool.tile([1, nbag], F32, tag="mml")
        nc.vector.scalar_tensor_tensor(mml[:], lt[:], -1.0, m[:],
                                       ALU.mult, ALU.add)
        lv = spool.tile([1, nbag], F32, tag="lv")
        nc.vector.tensor_add(lv[:], mml[:], sp[:])
        lp = spool.tile([1, 1], F32, tag="lp")
        nc.vector.tensor_reduce(lp[:], lv[:], mybir.AxisListType.X, ALU.add)
        lps = spool.tile([1, 1], F32, tag="lps")
        nc.vector.tensor_scalar(lps[:], lp[:], 1.0 / B, None, ALU.mult)
        nc.sync.dma_start(out=loss_o[:], in_=lps[:])

    nc.compile()
    return nc


class TileKernel:
    """Small helper: `with TileKernel(nc) as (tc, ctx):`"""

    def __init__(self, nc):
        self.nc = nc

    def __enter__(self):
        self.ctx = ExitStack()
        self.tc = self.ctx.enter_context(tile.TileContext(self.nc))
        return self.tc, self.ctx

    def __exit__(self, *exc):
        return self.ctx.__exit__(*exc)


# --------------------------------------------------------------------------
# Entry point
# --------------------------------------------------------------------------

def run(inputs, cfg=None, collective=True, trace=False, trace_kwargs=None):
    cfg = dict(FULL_CFG if cfg is None else cfg)
    layout = build_layout(inputs["token_ids"], inputs["segment_ids"], cfg)
    nc = build_program(layout, cfg, collective=collective)
    maps = param_maps(inputs, layout, cfg)
    res = run_bass_kernel_spmd(
        nc, maps, list(range(cfg["NCORES"])),
        trace=trace, **(trace_kwargs or {}))

    ncores = cfg["NCORES"]
    B = cfg["B"]
    logits = np.zeros(B, np.float32)
    loss = np.float64(0.0)
    for c in range(ncores):
        out = res.results[c]
        core_bags = np.flatnonzero(layout["bag_core"] == c)
        pos = (layout["bag_win"][core_bags] * WIN
               + layout["bag_pos"][core_bags])
        logits[core_bags] = out["logits"][0, pos]
        loss += np.float64(out["loss_part"][0, 0])
    return (np.float32(loss), logits), res


def kernel(**inputs):
    (loss, logits), _ = run(inputs)
    return (loss, logits)


# revision 17
# speedup vs baseline: 1.6955x; 1.0066x over previous
"""Trainium2 Bass kernel for nn_BOW_model (embedding-bag mean -> FC -> BatchNorm1d
-> ReLU -> FC -> BCEWithLogits loss).

Self-contained: builds + runs an 8-core SPMD Bass program via
concourse.bass_utils.run_bass_kernel_spmd.

Strategy
--------
Data-parallel over bags. 4096 bags are assigned to 8 cores x 4 windows x 128
bags (greedy-balanced so per-(vocab-chunk, window) token counts are even across
cores; capacities are computed from the actual inputs, so one SPMD program fits
every core). Per core:

  1. dma_gather the core's token embedding rows from the replicated table in
     HBM into SBUF, 128 tokens per tile (token -> partition). int16 gather
     indices limit a single gather to 32768 table rows, so the vocab is
     processed in 4 chunks of 32768 rows. Gathers round-robin over the 4 SWDGE
     queues: each queue's descriptors are generated by a different Q7 core
     pair, which parallelizes descriptor generation 4x.
  2. For each 128-token tile, build the 0/1 matrix P[tok, slot] =
     (slot == bagslot[tok]) on the otherwise-idle Scalar engine
     (a1 = |iota - bagslot|; P = relu(1 - a1), integer-exact), then one PE
     matmul accumulates P.T @ G into the window's PSUM bank. The 1/count
     scaling is applied once per window during the PSUM->SBUF copy.
  3. Stream order is window-major, so each window's transpose (PE) overlaps
     the remaining gathers. Then h_preT = W1T.T @ bowT + b1, partial
     sum/sumsq over the core's bags.
  4. AllReduce (8 cores) of the 4KB partial stats -> global BatchNorm mean/var
     on device; relu(a*h+c) on ACT; logits via M=1 matmuls; stable BCE via
     ln(1+exp(-|l|)); per-core partial loss.

Host work is limited to index preprocessing/sharding, parameter re-layout,
unsharding the logits, and summing the 8 partial losses.
"""

import os
import sys
from contextlib import ExitStack

import numpy as np

try:
    import concourse.bass  # noqa: F401  (present on default PYTHONPATH)
except ImportError:  # pragma: no cover - fallback for bare environments
    sys.path.insert(0, "/opt/trn_rl_repo")

import concourse.bass as bass  # noqa: E402
import concourse.tile as tile  # noqa: E402
from concourse import bacc, library_config, mybir  # noqa: E402
from concourse.bass_utils import run_bass_kernel_spmd  # noqa: E402

F32 = mybir.dt.float32
F32R = mybir.dt.float32r
BF16 = mybir.dt.bfloat16
I16 = mybir.dt.int16
ALU = mybir.AluOpType
ACT = mybir.ActivationFunctionType

# Problem constants (full-size problem; dev harnesses may pass a custom cfg).
FULL_CFG = dict(
    B=4096,          # bags
    V=100000,        # vocab
    D=512,           # hidden dim (must be 512: 4 slices of 128)
    EPS=1e-5,
    NCORES=8,
    NWIN=4,          # windows (128-bag groups) per core; B == NCORES*NWIN*128
    CHUNK_BITS=15,   # gather chunk = 32768 rows (int16 index limit)
    BLOCK_TILES=8,   # 128-token tiles per dma_gather call
    NQUEUES=4,       # SWDGE queues used round-robin by the gathers
    F32R=True,       # fp32r (1 cyc/row) vs fp32 (4 cyc/row) matmuls
    BF16=False,      # gather/convolve the embedding table in bf16
)

WIN = 128
TILE = 128


# --------------------------------------------------------------------------
# Host-side layout
# --------------------------------------------------------------------------

def build_layout(token_ids, segment_ids, cfg):
    """Compute the sharded token layout from the actual inputs.

    Returns a dict with per-core device arrays and the uniform capacities
    (program structure) shared by all cores.
    """
    B, V = cfg["B"], cfg["V"]
    ncores, nwin = cfg["NCORES"], cfg["NWIN"]
    chunk_bits = cfg["CHUNK_BITS"]
    chunk = 1 << chunk_bits
    nchunk = (V + chunk - 1) >> chunk_bits
    nslot = ncores * nwin
    assert B == nslot * WIN, "bag count must exactly fill cores*windows*128"

    tok = np.asarray(token_ids).astype(np.int64).ravel()
    seg = np.asarray(segment_ids).astype(np.int64).ravel()
    T = tok.shape[0]

    counts = np.bincount(seg, minlength=B)
    tok_chunk = tok >> chunk_bits
    # per-bag, per-chunk token counts (for balancing)
    bc = np.bincount(seg * nchunk + tok_chunk, minlength=B * nchunk).reshape(
        B, nchunk
    ).astype(np.int64)

    # Greedy: assign bags (largest first) to the (core,window) slot that
    # minimizes the resulting max normalized per-chunk load.
    order = np.argsort(-counts, kind="stable")
    slot_load = np.zeros((nslot, nchunk), np.float64)
    slot_n = np.zeros(nslot, np.int64)
    bag_slot = np.empty(B, np.int64)
    bag_pos = np.empty(B, np.int64)
    mean_c = np.maximum(bc.sum(axis=0) / nslot, 1.0)  # target per-slot load
    for b in order:
        avail = slot_n < WIN
        cand = (slot_load[avail] + bc[b]) / mean_c
        score = cand.max(axis=1) + 1e-4 * cand.sum(axis=1)
        s = np.flatnonzero(avail)[int(np.argmin(score))]
        bag_slot[b] = s
        bag_pos[b] = slot_n[s]
        slot_n[s] += 1
        slot_load[s] += bc[b]
    assert (slot_n == WIN).all()

    bag_core = bag_slot // nwin
    bag_win = bag_slot % nwin

    tok_core = bag_core[seg]
    tok_win = bag_win[seg]
    tok_pos = bag_pos[seg]                       # slot position within window
    tok_within = (tok & (chunk - 1)).astype(np.int64)

    # group key = (core, window, chunk) -- WINDOW-MAJOR stream order, so each
    # window's epilogue can overlap the remaining gathers.
    gkey = (tok_core * nwin + tok_win) * nchunk + tok_chunk
    gcnt = np.bincount(gkey, minlength=ncores * nwin * nchunk).reshape(
        ncores, nwin, nchunk
    )
    caps = gcnt.max(axis=0)                      # [nwin, nchunk]
    caps = ((caps + TILE - 1) // TILE) * TILE    # round to whole tiles
    ntok_cap = int(caps.sum())
    n_tiles = ntok_cap // TILE

    # group base offsets in the per-core token stream (window-major)
    gbase = np.zeros((nwin, nchunk), np.int64)
    off = 0
    for w in range(nwin):
        for c in range(nchunk):
            gbase[w, c] = off
            off += int(caps[w, c])

    # position of each token in its core's stream
    sort_idx = np.lexsort((np.arange(T), gkey))  # stable by group
    sorted_gkey = gkey[sort_idx]
    grp_start = np.zeros(ncores * nwin * nchunk + 1, np.int64)
    np.cumsum(np.bincount(sorted_gkey, minlength=ncores * nwin * nchunk),
              out=grp_start[1:])
    rank = np.arange(T) - grp_start[sorted_gkey]
    wn = (sorted_gkey // nchunk) % nwin
    ck = sorted_gkey % nchunk
    tok_stream_pos = np.empty(T, np.int64)
    tok_stream_pos[sort_idx] = gbase[wn, ck] + rank

    # fill per-core buffers
    idx_buf = np.zeros((ncores, ntok_cap), np.int16)
    slot_buf = np.full((ncores, ntok_cap), 1000.0, np.float32)
    idx_buf[tok_core, tok_stream_pos] = tok_within.astype(np.int16)
    slot_buf[tok_core, tok_stream_pos] = tok_pos.astype(np.float32)

    # device layouts
    # idx: token j at [j%16, j//16], replicated over the 8 groups of 16 rows
    idx_dev = np.tile(
        idx_buf.reshape(ncores, ntok_cap // 16, 16).transpose(0, 2, 1), (1, 8, 1)
    ).copy()                                              # [ncores,128,ntok/16]
    slot_dev = slot_buf.reshape(
        ncores, n_tiles, TILE).transpose(0, 2, 1).copy()  # [ncores,128,n_tiles]
    negslot_dev = -slot_dev
    stepbias_dev = 60.0 * (0.5 - slot_dev)   # sigmoid-step bias, K=60

    # per-(window, slot) 1/count, fp32 exact: [ncores, 128, nwin]
    invc_win = np.ones((ncores, WIN, nwin), np.float32)
    invc_win[bag_core, bag_pos, bag_win] = (
        1.0 / np.maximum(counts, 1.0)).astype(np.float32)
    # Transpose-matmul moving operand, one per window: recovers per-slot sums
    # from the cumulative-over-slot PSUM (difference) and applies 1/count, all
    # inside the existing PE transpose: dmat[w][p, j] = invc[j]*(d(p==j) -
    # d(p==j-1)).
    base = np.eye(WIN, dtype=np.float32) - np.eye(WIN, k=1, dtype=np.float32)
    dmat = np.empty((ncores, nwin, WIN, WIN), np.float32)
    for c in range(ncores):
        for w in range(nwin):
            dmat[c, w] = base * invc_win[c, :, w][None, :]

    return dict(
        caps=caps, nchunk=nchunk, chunk=chunk, ntok_cap=ntok_cap,
        n_tiles=n_tiles, idx_dev=idx_dev, negslot_dev=negslot_dev,
        slot_dev=slot_dev, stepbias_dev=stepbias_dev, invc_win=invc_win,
        dmat=dmat, bag_core=bag_core, bag_win=bag_win,
        bag_pos=bag_pos,
    )


def param_maps(inputs, layout, cfg):
    """Per-core input maps (device tensors) from full inputs + layout."""
    D, nwin = cfg["D"], cfg["NWIN"]
    ncores = cfg["NCORES"]
    njs = D // 128
    emb = np.asarray(inputs["emb"], np.float32)
    if cfg.get("BF16"):
        import ml_dtypes
        emb = emb.astype(ml_dtypes.bfloat16)
    emb = np.ascontiguousarray(emb)
    W1 = np.asarray(inputs["W1"], np.float32)
    w1t = np.ascontiguousarray(W1.T)                       # [d, j]
    b1c = np.asarray(inputs["b1"], np.float32).reshape(njs, 128).T.copy()
    gammac = np.asarray(inputs["gamma"], np.float32).reshape(njs, 128).T.copy()
    betac = np.asarray(inputs["beta"], np.float32).reshape(njs, 128).T.copy()
    w2c = np.asarray(inputs["W2"], np.float32).reshape(njs, 128).T.copy()
    b2c = np.asarray(inputs["b2"], np.float32).reshape(1, 1)
    t_full = np.asarray(inputs["t"], np.float32)

    iota_c = np.tile(np.arange(128, dtype=np.float32), (128, 1))

    bag_core = layout["bag_core"]
    bag_win = layout["bag_win"]
    bag_pos = layout["bag_pos"]

    maps = []
    for c in range(ncores):
        sel = bag_core == c
        tvec = np.zeros((1, nwin * WIN), np.float32)
        tvec[0, bag_win[sel] * WIN + bag_pos[sel]] = t_full[sel]
        maps.append(dict(
            emb=emb,
            idx=layout["idx_dev"][c],
            negslot=layout["negslot_dev"][c],
            posslot=layout["slot_dev"][c],
            stepbias=layout["stepbias_dev"][c],
            w1t=w1t, b1c=b1c, gammac=gammac, betac=betac,
            w2c=w2c, b2c=b2c, tvec=tvec,
            iota=iota_c, dmat=layout["dmat"][c],
        ))
    return maps


# --------------------------------------------------------------------------
# Device program
# --------------------------------------------------------------------------

def build_program(layout, cfg, collective=True):
    B, V, D = cfg["B"], cfg["V"], cfg["D"]
    ncores, nwin = cfg["NCORES"], cfg["NWIN"]
    chunk = layout["chunk"]
    nchunk = layout["nchunk"]
    caps = layout["caps"]                 # [nwin, nchunk]
    ntok_cap = layout["ntok_cap"]
    n_tiles = layout["n_tiles"]
    block_tiles = cfg["BLOCK_TILES"]
    nqueues = cfg.get("NQUEUES", 1)
    njs = D // 128
    nds = D // 128
    nbag = nwin * WIN                     # bags per core (free axis of h)
    mm = F32R if cfg.get("F32R") else F32  # h-chain matmul operand dtype
    gdt = BF16 if cfg.get("BF16") else mm  # gather/P matmul operand dtype

    nc = bacc.Bacc("TRN2", target_bir_lowering=False, debug=True,
                   num_devices=ncores, num_swdge_queues=nqueues)

    emb = nc.declare_dram_parameter("emb", [V, D], gdt, isOutput=False)
    idx = nc.declare_dram_parameter("idx", [128, ntok_cap // 16], I16, False)
    negslot = nc.declare_dram_parameter("negslot", [128, n_tiles], F32, False)
    posslot = nc.declare_dram_parameter("posslot", [128, n_tiles], F32, False)
    stepbias = nc.declare_dram_parameter("stepbias", [128, n_tiles], F32,
                                         False)
    dmat_in = nc.declare_dram_parameter("dmat", [nwin, WIN, WIN], F32, False)
    w1t = nc.declare_dram_parameter("w1t", [D, D], mm, False)
    b1c = nc.declare_dram_parameter("b1c", [128, njs], F32, False)
    gammac = nc.declare_dram_parameter("gammac", [128, njs], F32, False)
    betac = nc.declare_dram_parameter("betac", [128, njs], F32, False)
    w2c = nc.declare_dram_parameter("w2c", [128, njs], mm, False)
    b2c = nc.declare_dram_parameter("b2c", [1, 1], F32, False)
    tvec = nc.declare_dram_parameter("tvec", [1, nbag], F32, False)
    iota_in = nc.declare_dram_parameter("iota", [128, 128], F32, False)

    logits_o = nc.declare_dram_parameter("logits", [1, nbag], F32, isOutput=True)
    loss_o = nc.declare_dram_parameter("loss_part", [1, 1], F32, isOutput=True)
    stats_o = nc.declare_dram_parameter("stats", [128, 2 * njs], F32,
                                        isOutput=True)

    with TileKernel(nc) as (tc, ctx):
        nc.gpsimd.load_library(library_config.mlp)
        cpool = ctx.enter_context(tc.tile_pool(name="const", bufs=1))
        gpool = ctx.enter_context(tc.tile_pool(name="g", bufs=6))
        ppool = ctx.enter_context(tc.tile_pool(name="p", bufs=4))
        spool = ctx.enter_context(tc.tile_pool(name="small", bufs=1))
        bow_ps = ctx.enter_context(
            tc.tile_pool(name="bowps", bufs=nwin, space="PSUM"))
        aux_ps = ctx.enter_context(tc.tile_pool(name="auxps", bufs=2,
                                                space="PSUM"))
        h_ps = ctx.enter_context(tc.tile_pool(name="hps", bufs=2, space="PSUM"))
        hnp = ctx.enter_context(tc.tile_pool(name="hnp", bufs=2))
        dram = ctx.enter_context(tc.tile_pool(name="dram", bufs=1,
                                              space="DRAM"))

        # ---- resident loads ------------------------------------------------
        def load(name_ap, shape, dtype=F32):
            t = cpool.tile(shape, dtype, name=f"ld_{name_ap.name}",
                           tag=f"ld_{name_ap.name}")
            nc.sync.dma_start(out=t[:], in_=name_ap[:])
            return t

        # idx loads split so the first gather can start immediately
        ncol = ntok_cap // 16
        c0 = min(block_tiles * (TILE // 16), ncol)
        idx_sb = cpool.tile([128, ncol], I16, name="idx_sb", tag="idx_sb")
        nc.sync.dma_start(out=idx_sb[:, 0:c0], in_=idx[:, 0:c0])
        pb_mode = cfg.get("PBUILD", "step")
        if pb_mode == "step":
            sbias_sb = load(stepbias, [128, n_tiles])
        else:
            nslot_sb = load(negslot, [128, n_tiles])
            pslot_sb = load(posslot, [128, n_tiles])
            sbias_sb = load(stepbias, [128, n_tiles])
        if c0 < ncol:
            nc.sync.dma_start(out=idx_sb[:, c0:ncol], in_=idx[:, c0:ncol])
        iota_sb = load(iota_in, [128, 128])
        dmat_sb = []
        for w in range(nwin):
            dm = cpool.tile([128, WIN], F32, tag=f"dmat{w}", name=f"dmat{w}")
            nc.sync.dma_start(out=dm[:], in_=dmat_in[w])
            dmat_sb.append(dm)
        b1_sb = load(b1c, [128, njs])
        gam_sb = load(gammac, [128, njs])
        bet_sb = load(betac, [128, njs])
        w2_sb = load(w2c, [128, njs], mm)
        b2_sb = load(b2c, [1, 1])
        t_sb = load(tvec, [1, nbag])
        # W1T as nds tiles of [128 d, D j]
        w1t_sb = []
        for ds in range(nds):
            w = cpool.tile([128, D], mm, tag=f"w1t{ds}", name=f"w1t{ds}")
            nc.sync.dma_start(out=w[:], in_=w1t[ds * 128:(ds + 1) * 128, :])
            w1t_sb.append(w)

        bowT_sb = [spool.tile([128, nbag], mm, tag=f"bowT{ds}",
                              name=f"bowT{ds}")
                   for ds in range(nds)]
        h_sb = [spool.tile([128, nbag], F32, tag=f"h{js}", name=f"h{js}")
                for js in range(njs)]
        # per-(js, window) partial sums: [128, js*nwin + w], summed at the end
        sp_sb = spool.tile([128, njs * nwin], F32, tag="sp1", name="sp1_sb")
        sq_sb = spool.tile([128, njs * nwin], F32, tag="sp2", name="sp2_sb")

        # ---- main loop: gather + P-matmul accumulate (window-major) --------
        gather_cnt = 0
        gtile_idx = 0
        for w in range(nwin):
            bow = bow_ps.tile([128, D], F32, tag="bow", name=f"bow{w}")
            wtiles = int(caps[w].sum()) // TILE
            wseen = 0
            for c in range(nchunk):
                ctiles = int(caps[w][c]) // TILE
                if ctiles == 0:
                    continue
                base_row = c * chunk
                rows = min(chunk, V - base_row)
                done = 0
                while done < ctiles:
                    bt = min(block_tiles, ctiles - done)
                    g = gpool.tile([128, bt, D], gdt, tag="g")
                    col0 = (gtile_idx + done) * (TILE // 16)
                    nc.gpsimd.dma_gather(
                        out_ap=g[:],
                        in_ap=emb[base_row:base_row + rows, :],
                        idxs_ap=idx_sb[:, col0:col0 + bt * (TILE // 16)],
                        num_idxs=bt * TILE,
                        num_idxs_reg=bt * TILE,
                        elem_size=D,
                        queue_num=gather_cnt % nqueues,
                    )
                    gather_cnt += 1
                    for k in range(bt):
                        ti = gtile_idx + done + k
                        # P[tok, slot] = (slot == bagslot[tok]), integer-exact.
                        # "dve": one is_equal against the iota tile.
                        # "act": a1 = |iota - bagslot|; P = relu(1 - a1) on
                        #        the Scalar engine. "mix" alternates.
                        p = ppool.tile([128, 128], gdt, tag="p")
                        pb = cfg.get("PBUILD", "step")
                        if pb == "mix":
                            pb = "dve" if ti % 2 == 0 else "act"
                        if pb == "step":
                            # S[tok, f] = 1 iff f >= bagslot[tok]: sigmoid
                            # saturates exactly at +-30. The matmul then
                            # accumulates cumulative-over-slot sums; the
                            # window epilogue takes the adjacent difference.
                            nc.scalar.activation(p[:], iota_sb[:], ACT.Sigmoid,
                                                 scale=60.0,
                                                 bias=sbias_sb[:, ti:ti + 1])
                        elif pb == "act":
                            a1 = ppool.tile([128, 128], F32, tag="a1",
                                            name=f"a1_{ti}")
                            nc.scalar.activation(a1[:], iota_sb[:], ACT.Abs,
                                                 bias=nslot_sb[:, ti:ti + 1])
                            nc.scalar.activation(p[:], a1[:], ACT.Relu,
                                                 scale=-1.0, bias=1.0)
                        else:
                            nc.vector.tensor_scalar(
                                p[:], iota_sb[:], pslot_sb[:, ti:ti + 1],
                                None, ALU.is_equal)
                        nc.tensor.matmul(
                            bow[:], p[:], g[:, k, :],
                            start=(wseen == 0),
                            stop=(wseen == wtiles - 1))
                        wseen += 1
                    done += bt
                gtile_idx += ctiles

            # ---- window epilogue -------------------------------------------
            # The PE transpose's moving operand is dmat[w] (not identity): it
            # simultaneously transposes, takes the adjacent difference that
            # turns the cumulative-over-slot PSUM into per-slot sums (step
            # P-build), and applies the per-bag 1/count.
            bsb = spool.tile([128, D], F32, tag="bow_sb", name=f"bow_sb{w}",
                             bufs=2)
            nc.vector.tensor_copy(bsb[:], bow[:])
            for ds in range(nds):
                tp = aux_ps.tile([128, 128], F32, tag="tp")
                nc.tensor.matmul(tp[:], bsb[:, ds * 128:(ds + 1) * 128],
                                 dmat_sb[w][:], start=True, stop=True)
                nc.vector.tensor_copy(bowT_sb[ds][:, w * WIN:(w + 1) * WIN],
                                      tp[:])
            # h slice for this window's bags + partial stats (overlaps the
            # remaining gathers)
            for js in range(njs):
                hp = h_ps.tile([128, WIN], F32, tag="hps")
                for ds in range(nds):
                    nc.tensor.matmul(
                        hp[:],
                        w1t_sb[ds][:, js * 128:(js + 1) * 128],
                        bowT_sb[ds][:, w * WIN:(w + 1) * WIN],
                        start=(ds == 0), stop=(ds == nds - 1))
                hcol = h_sb[js][:, w * WIN:(w + 1) * WIN]
                nc.vector.tensor_scalar_add(hcol, hp[:], b1_sb[:, js:js + 1])
                col = js * nwin + w
                nc.vector.tensor_reduce(sp_sb[:, col:col + 1], hcol,
                                        mybir.AxisListType.X, ALU.add)
                sq = spool.tile([128, WIN], F32, tag="sqw", bufs=2,
                                name=f"sq_{w}_{js}")
                nc.vector.tensor_mul(sq[:], hcol, hcol)
                nc.vector.tensor_reduce(sq_sb[:, col:col + 1], sq[:],
                                        mybir.AxisListType.X, ALU.add)

        # ---- combine per-window partial stats ------------------------------
        s_sb = spool.tile([128, 2 * njs], F32, tag="s12")
        nc.vector.tensor_reduce(
            s_sb[:, 0:njs],
            sp_sb[:].rearrange("p (j w) -> p j w", w=nwin),
            mybir.AxisListType.X, ALU.add)
        nc.vector.tensor_reduce(
            s_sb[:, njs:2 * njs],
            sq_sb[:].rearrange("p (j w) -> p j w", w=nwin),
            mybir.AxisListType.X, ALU.add)

        # ---- global stats (AllReduce) --------------------------------------
        if collective and ncores > 1:
            cc_in = dram.tile([128, 2 * njs], F32)
            cc_out = dram.tile([128, 2 * njs], F32)
            nc.sync.dma_start(out=cc_in[:], in_=s_sb[:])
            nc.gpsimd.collective_compute(
                "AllReduce", ALU.add,
                replica_groups=[list(range(ncores))],
                ins=[cc_in.opt()], outs=[cc_out.opt()])
            sg_sb = spool.tile([128, 2 * njs], F32, tag="sg")
            nc.sync.dma_start(out=sg_sb[:], in_=cc_out[:])
        else:
            sg_sb = s_sb
        nc.sync.dma_start(out=stats_o[:], in_=sg_sb[:])

        mu = spool.tile([128, njs], F32, tag="mu")
        nc.vector.tensor_scalar(mu[:], sg_sb[:, 0:njs], 1.0 / B, None, ALU.mult)
        ex2 = spool.tile([128, njs], F32, tag="ex2")
        nc.vector.tensor_scalar(ex2[:], sg_sb[:, njs:2 * njs], 1.0 / B, None,
                                ALU.mult)
        mu2 = spool.tile([128, njs], F32, tag="mu2")
        nc.vector.tensor_mul(mu2[:], mu[:], mu[:])
        var = spool.tile([128, njs], F32, tag="var")
        nc.vector.scalar_tensor_tensor(var[:], mu2[:], -1.0, ex2[:],
                                       ALU.mult, ALU.add)
        vpe = spool.tile([128, njs], F32, tag="vpe")
        nc.vector.tensor_scalar_add(vpe[:], var[:], float(cfg["EPS"]))
        sd = spool.tile([128, njs], F32, tag="sd")
        nc.scalar.activation(sd[:], vpe[:], ACT.Sqrt)
        rstd = spool.tile([128, njs], F32, tag="rstd")
        nc.vector.reciprocal(rstd[:], sd[:])
        a_sb = spool.tile([128, njs], F32, tag="a")
        nc.vector.tensor_mul(a_sb[:], gam_sb[:], rstd[:])
        mua = spool.tile([128, njs], F32, tag="mua")
        nc.vector.tensor_mul(mua[:], mu[:], a_sb[:])
        c_sb = spool.tile([128, njs], F32, tag="c")
        nc.vector.scalar_tensor_tensor(c_sb[:], mua[:], -1.0, bet_sb[:],
                                       ALU.mult, ALU.add)

        # ---- normalize + relu + logits -------------------------------------
        # reuse the transpose slots (all 8 PSUM banks accounted: 4 bow + 2 tp
        # + 2 h)
        lg_ps = aux_ps.tile([1, nbag], F32, tag="tp")
        for js in range(njs):
            hn = hnp.tile([128, nbag], mm, tag="hn", name=f"hn{js}")
            nc.scalar.activation(hn[:], h_sb[js][:], ACT.Relu,
                                 scale=a_sb[:, js:js + 1],
                                 bias=c_sb[:, js:js + 1])
            nc.tensor.matmul(lg_ps[:], w2_sb[:, js:js + 1], hn[:],
                             start=(js == 0), stop=(js == njs - 1))
        lg = spool.tile([1, nbag], F32, tag="lg")
        nc.vector.tensor_scalar_add(lg[:], lg_ps[:], b2_sb[0:1, 0:1])
        nc.sync.dma_start(out=logits_o[:], in_=lg[:])

        # ---- BCE loss partial ----------------------------------------------
        m = spool.tile([1, nbag], F32, tag="m")
        nc.vector.tensor_scalar_max(m[:], lg[:], 0.0)
        lt = spool.tile([1, nbag], F32, tag="lt")
        nc.vector.tensor_mul(lt[:], lg[:], t_sb[:])
        al = spool.tile([1, nbag], F32, tag="al")
        nc.scalar.activation(al[:], lg[:], ACT.Abs)
        u = spool.tile([1, nbag], F32, tag="u")
        nc.scalar.activation(u[:], al[:], ACT.Exp, scale=-1.0)
        up1 = spool.tile([1, nbag], F32, tag="up1")
        nc.vector.tensor_scalar_add(up1[:], u[:], 1.0)
        sp = spool.tile([1, nbag], F32, tag="sp")
        nc.scalar.activation(sp[:], up1[:], ACT.Ln)
        mml = spool.tile([1, nbag], F32, tag="mml")
        nc.vector.scalar_tensor_tensor(mml[:], lt[:], -1.0, m[:],
                                       ALU.mult, ALU.add)
        lv = spool.tile([1, nbag], F32, tag="lv")
        nc.vector.tensor_add(lv[:], mml[:], sp[:])
        lp = spool.tile([1, 1], F32, tag="lp")
        nc.vector.tensor_reduce(lp[:], lv[:], mybir.AxisListType.X, ALU.add)
        lps = spool.tile([1, 1], F32, tag="lps")
        nc.vector.tensor_scalar(lps[:], lp[:], 1.0 / B, None, ALU.mult)
        nc.sync.dma_start(out=loss_o[:], in_=lps[:])

    nc.compile()
    return nc


class TileKernel:
    """Small helper: `with TileKernel(nc) as (tc, ctx):`"""

    def __init__(self, nc):
        self.nc = nc

    def __enter__(self):
        self.ctx = ExitStack()
        self.tc = self.ctx.enter_context(tile.TileContext(self.nc))
        return self.tc, self.ctx

    def __exit__(self, *exc):
        return self.ctx.__exit__(*exc)


# --------------------------------------------------------------------------
# Entry point
# --------------------------------------------------------------------------

def run(inputs, cfg=None, collective=True, trace=False, trace_kwargs=None):
    cfg = dict(FULL_CFG if cfg is None else cfg)
    layout = build_layout(inputs["token_ids"], inputs["segment_ids"], cfg)
    nc = build_program(layout, cfg, collective=collective)
    maps = param_maps(inputs, layout, cfg)
    res = run_bass_kernel_spmd(
        nc, maps, list(range(cfg["NCORES"])),
        trace=trace, **(trace_kwargs or {}))

    ncores = cfg["NCORES"]
    B = cfg["B"]
    logits = np.zeros(B, np.float32)
    loss = np.float64(0.0)
    for c in range(ncores):
        out = res.results[c]
        core_bags = np.flatnonzero(layout["bag_core"] == c)
        pos = (layout["bag_win"][core_bags] * WIN
               + layout["bag_pos"][core_bags])
        logits[core_bags] = out["logits"][0, pos]
        loss += np.float64(out["loss_part"][0, 0])
    return (np.float32(loss), logits), res


def kernel(**inputs):
    (loss, logits), _ = run(inputs)
    return (loss, logits)
